# revision 5
# baseline (speedup 1.0000x reference)
"""Variant G2: G + bf16 output (host-upcast) + out-DMA on sync queue.

Per-core SwiGLU expert MLP (one expert per NeuronCore, 8 cores):
  out[4096, 2048] = (silu(x@wg) * (x@wu)) @ wd

Schedule is Variant F's interleave (mm3 of group g-1 emitted in bursts
between group g's mm1/mm2 runs) with:
  - all inputs cast to bf16 on host, in layouts giving >=4KB contiguous
    DMA rows per partition (HWDGE sync/scalar queues; no SWDGE casts)
  - weights DMA'd in hh slices so mm1 starts after ~1MB is resident
  - native AF.Silu (one Act op instead of sigmoid + DVE mul)
  - mm3 bursts sized [8,8,8,8,12] per token-block so a new token-block's
    first matmul always starts right after an mm1/mm2 run, giving the
    previous block's PSUM->SBUF copies a full run to drain
"""

import numpy as np

NUM_EXPERTS = 8
DIM = 2048
HIDDEN = 1408
T_E = 4096

P = 128
TN = 512
NG = T_E // TN          # 8 token groups per expert
DC = DIM // P           # 16 contraction chunks (mm1/mm2)
HC = HIDDEN // P        # 11 hidden chunks
NDO = DIM // TN         # 4 output column chunks (mm3)
TB = TN // P            # 4 token blocks of 128 per group

# mm3 burst sizes at the 22 interleave positions per group (2 per hh slot).
# Token-block boundaries (idx 0, 44, 88, 132) land at position starts.
_BURSTS = [8, 8, 8, 8, 12] * TB + [0, 0]

_nc_cache = []


def _build_program(n_reps=1, phase=None):
    import concourse.mybir as mybir
    import concourse.tile as tile
    from concourse import bacc

    fp32 = mybir.dt.float32
    bf16 = mybir.dt.bfloat16
    AF = mybir.ActivationFunctionType

    nc = bacc.Bacc("TRN2", target_bir_lowering=False, debug=False)

    # Host-prepared layouts (see _make_in_maps):
    #   x3:  [NG*P, DC*TN]  x3[g*P+p, c*TN+t] = x[g*TN+t, c*P+p]
    #   wg3: [HC*P, DC*P]   wg3[h*P+p, c*P+j] = wg[c*P+p, h*P+j]
    #   wu3: same as wg3
    #   wd2: [HIDDEN, DIM]  (natural layout, bf16)
    x3 = nc.dram_tensor("x3", [NG * P, DC * TN], bf16, kind="ExternalInput")
    wg3 = nc.dram_tensor("wg3", [HC * P, DC * P], bf16, kind="ExternalInput")
    wu3 = nc.dram_tensor("wu3", [HC * P, DC * P], bf16, kind="ExternalInput")
    wd2 = nc.dram_tensor("wd2", [HIDDEN, DIM], bf16, kind="ExternalInput")
    out = nc.dram_tensor("out", [T_E, DIM], bf16, kind="ExternalOutput")

    with tile.TileContext(nc) as tc:
        with (
            tc.tile_pool(name="wpool", bufs=1) as wpool,
            tc.tile_pool(name="xpool", bufs=2) as xpool,
            tc.tile_pool(name="hpool", bufs=2) as hpool,
            tc.tile_pool(name="spool", bufs=1) as spool,
            tc.tile_pool(name="opool", bufs=2) as opool,
            tc.tile_pool(name="psum", bufs=2, space="PSUM") as psum_pool,
        ):
            wg_sb = wpool.tile([P, HC, DC, P], bf16, tag="wg")
            wu_sb = wpool.tile([P, HC, DC, P], bf16, tag="wu")
            wd_sb = wpool.tile([P, HC, DIM], bf16, tag="wd")
            xt0_sb = xpool.tile([P, DC, TN], bf16, tag="xt")

            # Prologue order: first slot's weights, first group's x, then
            # the rest (weights on sync queue, x on scalar queue).
            nc.sync.dma_start(out=wg_sb[:, 0], in_=wg3[0:P, :])
            nc.sync.dma_start(out=wu_sb[:, 0], in_=wu3[0:P, :])
            nc.scalar.dma_start(out=xt0_sb, in_=x3[0:P, :])
            for h in range(1, HC):
                nc.sync.dma_start(out=wg_sb[:, h], in_=wg3[h * P:(h + 1) * P, :])
                nc.sync.dma_start(out=wu_sb[:, h], in_=wu3[h * P:(h + 1) * P, :])
            for h in range(HC):
                nc.sync.dma_start(out=wd_sb[:, h], in_=wd2[h * P:(h + 1) * P, :])

            # mm3 emission state for the previous group.
            st = {"po": None, "ot": None, "idx": 0}

            def emit_mm3(size, pht, pgrp):
                for idx in range(st["idx"], st["idx"] + size):
                    tb, r = divmod(idx, HC * NDO)
                    hh3, do = divmod(r, NDO)
                    if r == 0:
                        st["po"] = psum_pool.tile(
                            [P, NDO, TN], fp32, tag="po", bufs=1, name="po"
                        )
                        st["ot"] = opool.tile([P, DIM], bf16, tag="ot", name="ot")
                    nc.tensor.matmul(
                        st["po"][:, do, :],
                        pht[:, hh3, tb * P:(tb + 1) * P],
                        wd_sb[:, hh3, do * TN:(do + 1) * TN],
                        start=(hh3 == 0),
                        stop=(hh3 == HC - 1),
                    )
                    if r == HC * NDO - 1:
                        for do2 in range(NDO):
                            nc.vector.tensor_copy(
                                st["ot"][:, do2 * TN:(do2 + 1) * TN],
                                st["po"][:, do2, :],
                            )
                        t0 = pgrp * TN + tb * P
                        nc.sync.dma_start(out=out[t0:t0 + P, :], in_=st["ot"])
                st["idx"] += size

            prev = None  # (ht_sb, group index)
            for g in range(NG + 1):
                if g < NG:
                    if g == 0:
                        xt_sb = xt0_sb
                    else:
                        xt_sb = xpool.tile([P, DC, TN], bf16, tag="xt")
                        nc.scalar.dma_start(
                            out=xt_sb, in_=x3[g * P:(g + 1) * P, :]
                        )
                    ht_sb = hpool.tile([P, HC, TN], bf16, tag="ht")
                st["idx"] = 0

                for hh in range(HC):
                    if g < NG:
                        pg = psum_pool.tile([P, TN], fp32, tag="pg")
                        pu = psum_pool.tile([P, TN], fp32, tag="pu")
                        for c in range(DC):
                            nc.tensor.matmul(
                                pg,
                                wg_sb[:, hh, c, :],
                                xt_sb[:, c, :],
                                start=(c == 0),
                                stop=(c == DC - 1),
                            )
                        if prev is not None:
                            emit_mm3(_BURSTS[2 * hh], prev[0], prev[1])
                        for c in range(DC):
                            nc.tensor.matmul(
                                pu,
                                wu_sb[:, hh, c, :],
                                xt_sb[:, c, :],
                                start=(c == 0),
                                stop=(c == DC - 1),
                            )
                        sig = spool.tile([P, TN], fp32, tag="sig")
                        sil = spool.tile([P, TN], fp32, tag="sil")
                        nc.scalar.activation(sig, pg, AF.Sigmoid)
                        nc.vector.tensor_mul(sil, pg, sig)
                        nc.vector.tensor_mul(ht_sb[:, hh, :], sil, pu)
                        if prev is not None:
                            emit_mm3(_BURSTS[2 * hh + 1], prev[0], prev[1])
                    if g == NG and prev is not None:
                        emit_mm3(_BURSTS[2 * hh], prev[0], prev[1])
                        emit_mm3(_BURSTS[2 * hh + 1], prev[0], prev[1])

                prev = (ht_sb, g) if g < NG else None

    nc.compile()
    return nc


def _get_program():
    if not _nc_cache:
        _nc_cache.append(_build_program())
    return _nc_cache[0]


def _host_prep(x_e, w_gate_e, w_up_e, w_down_e):
    """Per-expert host-side layout prep + bf16 cast."""
    import ml_dtypes

    bf16 = ml_dtypes.bfloat16
    # x_e [T_E, DIM] -> [NG, TN, DC, P] -> [NG, P, DC, TN] -> 2D
    x3 = np.ascontiguousarray(
        x_e.reshape(NG, TN, DC, P).transpose(0, 3, 2, 1), dtype=bf16
    ).reshape(NG * P, DC * TN)
    # wg [DIM, HIDDEN] -> [DC, P, HC, P] -> [HC, P, DC, P] -> 2D
    wg3 = np.ascontiguousarray(
        w_gate_e.reshape(DC, P, HC, P).transpose(2, 1, 0, 3), dtype=bf16
    ).reshape(HC * P, DC * P)
    wu3 = np.ascontiguousarray(
        w_up_e.reshape(DC, P, HC, P).transpose(2, 1, 0, 3), dtype=bf16
    ).reshape(HC * P, DC * P)
    wd2 = np.ascontiguousarray(w_down_e, dtype=bf16)
    return {"x3": x3, "wg3": wg3, "wu3": wu3, "wd2": wd2}


def _make_in_maps(inputs):
    x = np.asarray(inputs["x"], dtype=np.float32)
    w_gate = np.asarray(inputs["w_gate"], dtype=np.float32)
    w_up = np.asarray(inputs["w_up"], dtype=np.float32)
    w_down = np.asarray(inputs["w_down"], dtype=np.float32)
    xe = x.reshape(NUM_EXPERTS, T_E, DIM)
    return [
        _host_prep(xe[e], w_gate[e], w_up[e], w_down[e])
        for e in range(NUM_EXPERTS)
    ]


def kernel(x, num_tokens_per_expert, w_gate, w_up, w_down, **_ignored):
    from concourse.bass_utils import run_bass_kernel_spmd

    nc = _get_program()
    in_maps = _make_in_maps(
        {"x": x, "w_gate": w_gate, "w_up": w_up, "w_down": w_down}
    )
    res = run_bass_kernel_spmd(nc, in_maps, core_ids=list(range(NUM_EXPERTS)))
    outs = [np.asarray(r["out"]).astype(np.float32) for r in res.results]
    return np.concatenate(outs, axis=0)


# revision 7
# speedup vs baseline: 1.8952x; 1.8952x over previous
"""Variant G2: G + bf16 output (host-upcast) + out-DMA on sync queue.

Per-core SwiGLU expert MLP (one expert per NeuronCore, 8 cores):
  out[4096, 2048] = (silu(x@wg) * (x@wu)) @ wd

Schedule is Variant F's interleave (mm3 of group g-1 emitted in bursts
between group g's mm1/mm2 runs) with:
  - all inputs cast to bf16 on host, in layouts giving >=4KB contiguous
    DMA rows per partition (HWDGE sync/scalar queues; no SWDGE casts)
  - weights DMA'd in hh slices so mm1 starts after ~1MB is resident
  - native AF.Silu (one Act op instead of sigmoid + DVE mul)
  - mm3 bursts sized [8,8,8,8,12] per token-block so a new token-block's
    first matmul always starts right after an mm1/mm2 run, giving the
    previous block's PSUM->SBUF copies a full run to drain
"""

import numpy as np

NUM_EXPERTS = 8
DIM = 2048
HIDDEN = 1408
T_E = 4096

P = 128
TN = 512
NG = T_E // TN          # 8 token groups per expert
DC = DIM // P           # 16 contraction chunks (mm1/mm2)
HC = HIDDEN // P        # 11 hidden chunks
NDO = DIM // TN         # 4 output column chunks (mm3)
TB = TN // P            # 4 token blocks of 128 per group

# mm3 burst sizes at the 22 interleave positions per group (2 per hh slot).
# Token-block boundaries (idx 0, 44, 88, 132) land at position starts.
_BURSTS = [8, 8, 8, 8, 12] * TB + [0, 0]

_nc_cache = []


def _build_program(n_reps=1, phase=None):
    import concourse.mybir as mybir
    import concourse.tile as tile
    from concourse import bacc

    fp32 = mybir.dt.float32
    bf16 = mybir.dt.bfloat16
    AF = mybir.ActivationFunctionType

    nc = bacc.Bacc("TRN2", target_bir_lowering=False, debug=False)

    # Host-prepared layouts (see _make_in_maps):
    #   x3:  [NG*P, DC*TN]  x3[g*P+p, c*TN+t] = x[g*TN+t, c*P+p]
    #   wg3: [HC*P, DC*P]   wg3[h*P+p, c*P+j] = wg[c*P+p, h*P+j]
    #   wu3: same as wg3
    #   wd2: [HIDDEN, DIM]  (natural layout, bf16)
    x3 = nc.dram_tensor("x3", [NG * P, DC * TN], bf16, kind="ExternalInput")
    wg3 = nc.dram_tensor("wg3", [HC * P, DC * P], bf16, kind="ExternalInput")
    wu3 = nc.dram_tensor("wu3", [HC * P, DC * P], bf16, kind="ExternalInput")
    wd2 = nc.dram_tensor("wd2", [HIDDEN, DIM], bf16, kind="ExternalInput")
    out = nc.dram_tensor("out", [T_E, DIM], bf16, kind="ExternalOutput")

    with tile.TileContext(nc) as tc:
        with (
            tc.tile_pool(name="wpool", bufs=1) as wpool,
            tc.tile_pool(name="xpool", bufs=2) as xpool,
            tc.tile_pool(name="hpool", bufs=2) as hpool,
            tc.tile_pool(name="spool", bufs=1) as spool,
            tc.tile_pool(name="opool", bufs=2) as opool,
            tc.tile_pool(name="psum", bufs=2, space="PSUM") as psum_pool,
        ):
            wg_sb = wpool.tile([P, HC, DC, P], bf16, tag="wg")
            wu_sb = wpool.tile([P, HC, DC, P], bf16, tag="wu")
            wd_sb = wpool.tile([P, HC, DIM], bf16, tag="wd")
            xt0_sb = xpool.tile([P, DC, TN], bf16, tag="xt")

            # Prologue order: first slot's weights, first group's x, then
            # the rest (weights on sync queue, x on scalar queue).
            nc.sync.dma_start(out=wg_sb[:, 0], in_=wg3[0:P, :])
            nc.sync.dma_start(out=wu_sb[:, 0], in_=wu3[0:P, :])
            nc.scalar.dma_start(out=xt0_sb, in_=x3[0:P, :])
            for h in range(1, HC):
                nc.sync.dma_start(out=wg_sb[:, h], in_=wg3[h * P:(h + 1) * P, :])
                nc.sync.dma_start(out=wu_sb[:, h], in_=wu3[h * P:(h + 1) * P, :])
            for h in range(HC):
                nc.sync.dma_start(out=wd_sb[:, h], in_=wd2[h * P:(h + 1) * P, :])

            # mm3 emission state for the previous group.
            st = {"po": None, "ot": None, "idx": 0}

            def emit_mm3(size, pht, pgrp):
                for idx in range(st["idx"], st["idx"] + size):
                    tb, r = divmod(idx, HC * NDO)
                    hh3, do = divmod(r, NDO)
                    if r == 0:
                        st["po"] = psum_pool.tile(
                            [P, NDO, TN], fp32, tag="po", bufs=1, name="po"
                        )
                        st["ot"] = opool.tile([P, DIM], bf16, tag="ot", name="ot")
                    nc.tensor.matmul(
                        st["po"][:, do, :],
                        pht[:, hh3, tb * P:(tb + 1) * P],
                        wd_sb[:, hh3, do * TN:(do + 1) * TN],
                        start=(hh3 == 0),
                        stop=(hh3 == HC - 1),
                    )
                    if r == HC * NDO - 1:
                        for do2 in range(NDO):
                            nc.vector.tensor_copy(
                                st["ot"][:, do2 * TN:(do2 + 1) * TN],
                                st["po"][:, do2, :],
                            )
                        t0 = pgrp * TN + tb * P
                        nc.sync.dma_start(out=out[t0:t0 + P, :], in_=st["ot"])
                st["idx"] += size

            prev = None  # (ht_sb, group index)
            for g in range(NG + 1):
                if g < NG:
                    if g == 0:
                        xt_sb = xt0_sb
                    else:
                        xt_sb = xpool.tile([P, DC, TN], bf16, tag="xt")
                        nc.scalar.dma_start(
                            out=xt_sb, in_=x3[g * P:(g + 1) * P, :]
                        )
                    ht_sb = hpool.tile([P, HC, TN], bf16, tag="ht")
                st["idx"] = 0

                for hh in range(HC):
                    if g < NG:
                        pg = psum_pool.tile([P, TN], fp32, tag="pg")
                        pu = psum_pool.tile([P, TN], fp32, tag="pu")
                        for c in range(DC):
                            nc.tensor.matmul(
                                pg,
                                wg_sb[:, hh, c, :],
                                xt_sb[:, c, :],
                                start=(c == 0),
                                stop=(c == DC - 1),
                            )
                        if prev is not None:
                            emit_mm3(_BURSTS[2 * hh], prev[0], prev[1])
                        for c in range(DC):
                            nc.tensor.matmul(
                                pu,
                                wu_sb[:, hh, c, :],
                                xt_sb[:, c, :],
                                start=(c == 0),
                                stop=(c == DC - 1),
                            )
                        sig = spool.tile([P, TN], fp32, tag="sig")
                        sil = spool.tile([P, TN], fp32, tag="sil")
                        nc.scalar.activation(sig, pg, AF.Sigmoid)
                        nc.vector.tensor_mul(sil, pg, sig)
                        nc.vector.tensor_mul(ht_sb[:, hh, :], sil, pu)
                        if prev is not None:
                            emit_mm3(_BURSTS[2 * hh + 1], prev[0], prev[1])
                    if g == NG and prev is not None:
                        emit_mm3(_BURSTS[2 * hh], prev[0], prev[1])
                        emit_mm3(_BURSTS[2 * hh + 1], prev[0], prev[1])

                prev = (ht_sb, g) if g < NG else None

    nc.compile()
    return nc


def _get_program():
    if not _nc_cache:
        _nc_cache.append(_build_program())
    return _nc_cache[0]


def _host_prep(x_e, w_gate_e, w_up_e, w_down_e):
    """Per-expert host-side layout prep + bf16 cast."""
    import ml_dtypes

    bf16 = ml_dtypes.bfloat16
    # x_e [T_E, DIM] -> [NG, TN, DC, P] -> [NG, P, DC, TN] -> 2D
    x3 = np.ascontiguousarray(
        x_e.reshape(NG, TN, DC, P).transpose(0, 3, 2, 1), dtype=bf16
    ).reshape(NG * P, DC * TN)
    # wg [DIM, HIDDEN] -> [DC, P, HC, P] -> [HC, P, DC, P] -> 2D
    wg3 = np.ascontiguousarray(
        w_gate_e.reshape(DC, P, HC, P).transpose(2, 1, 0, 3), dtype=bf16
    ).reshape(HC * P, DC * P)
    wu3 = np.ascontiguousarray(
        w_up_e.reshape(DC, P, HC, P).transpose(2, 1, 0, 3), dtype=bf16
    ).reshape(HC * P, DC * P)
    wd2 = np.ascontiguousarray(w_down_e, dtype=bf16)
    return {"x3": x3, "wg3": wg3, "wu3": wu3, "wd2": wd2}


def _make_in_maps(inputs):
    x = np.asarray(inputs["x"], dtype=np.float32)
    w_gate = np.asarray(inputs["w_gate"], dtype=np.float32)
    w_up = np.asarray(inputs["w_up"], dtype=np.float32)
    w_down = np.asarray(inputs["w_down"], dtype=np.float32)
    xe = x.reshape(NUM_EXPERTS, T_E, DIM)
    return [
        _host_prep(xe[e], w_gate[e], w_up[e], w_down[e])
        for e in range(NUM_EXPERTS)
    ]


def kernel(x, num_tokens_per_expert, w_gate, w_up, w_down, **_ignored):
    from concourse.bass_utils import run_bass_kernel_spmd

    nc = _get_program()
    in_maps = _make_in_maps(
        {"x": x, "w_gate": w_gate, "w_up": w_up, "w_down": w_down}
    )
    res = run_bass_kernel_spmd(nc, in_maps, core_ids=list(range(NUM_EXPERTS)))
    outs = [np.asarray(r["out"]).astype(np.float32) for r in res.results]
    return np.concatenate(outs, axis=0)
